# revision 23
# baseline (speedup 1.0000x reference)
"""Multi-head causal self-attention (B=2, S=4096, D=512, H=8) on 8 trn2 cores.

Sharding: batch*heads = 16 (b,h) pairs -> 2 heads per core (head-parallel,
qkv weight columns sharded per head group). Zero cross-core communication.

Per-core kernel (heads h0=2g, h1=2g+1 stacked on partition halves):
  - inputs: xt = X[b].T  (512, 4096),  w = [Wq|Wk|Wv] head cols (512, 384)
  - QT/KT: (128, 4096) with partitions 0-63 = head0 dims, 64-127 = head1
  - V: natural layout per 128-row j-tile, with an appended ones column so
    the AV matmul also produces the softmax denominator (row 64 of out).
  - scores computed transposed (keys on partitions) so softmax sum comes
    from the ones column; exp on ScalarE with scale=1/8 folded in; causal
    mask via gpsimd affine_select on the exp'd diag tiles (the slice
    starts on the diagonal, so keep iff col_in_slice >= partition).
  - AV accumulated in PSUM over j-tiles; result O.T (65, 512) transposed
    back via PE transpose in 128-col blocks; normalized with per-partition
    reciprocal of the denominator column; DMA'd out as full 512B rows.

Schedule: xt is DMA'd in 8 column blocks so range-0 Q/K projection (block
0 only) starts immediately; V projection for j-tiles 4t+4..4t+7 and Q/K
projection for range t+1 are interleaved into range t's pair loop.  PSUM
is exactly 8 banks: scores 2x2, AV accumulators 2x1, and a shared 2x1
"aux" tag that rotates between projection outputs and tail transposes.
Within a pair, scores for pair p+1 are emitted BEFORE the AV of pair p so
the PE streams independent work while ScalarE exponentiates.  The final
range's last-pair AV is emitted per 128-column block so the transpose/
normalize/store tail pipelines with it instead of serializing after the
loop.
"""

import os
import sys

import numpy as np

for _p in ("/opt/trn_rl_repo", "/root/.axon_site/_ro/trn_rl_repo"):
    if os.path.isdir(_p) and _p not in sys.path:
        sys.path.append(_p)

import concourse.bass as bass
import concourse.tile as tile
from concourse import mybir
from concourse.masks import make_identity

F32 = mybir.dt.float32
F32R = mybir.dt.float32r
BF16 = mybir.dt.bfloat16

B, S, D, H = 2, 4096, 512, 8
HD = 64          # head dim
NHC = 2          # heads per core
P = 128          # partitions
KC = D // P      # 4 contraction chunks for the projection
IT = 512         # query-range width
NI = S // IT     # 8 query ranges
JT = 128         # key-tile width
NJ = S // JT     # 32 key tiles
SCALE = 1.0 / np.sqrt(HD)  # 0.125
# Schraudolph exp for bf16 via uint16 bit-pattern: for x = score*SCALE,
# bits = round(score * SCH_A + SCH_B) approximates bf16(exp(x)).  C=6.0
# calibrated for min max-rel-err (~3.5%), which washes out in the softmax
# average (validated: output max rel err unchanged at 0.0065).
SCH_A = float(SCALE * np.log2(np.e) * 128.0)
SCH_B = float(127.0 * 128.0 - 6.0)


def build_nc():
    nc = bass.Bass()
    xt = nc.declare_dram_parameter("xt", [D, S], BF16, isOutput=False)
    w = nc.declare_dram_parameter("w", [D, 3 * P], BF16, isOutput=False)
    out = nc.declare_dram_parameter("out", [S, NHC * HD], F32, isOutput=True)

    with tile.TileContext(nc) as tc:
        with (
            tc.tile_pool(name="singles", bufs=1) as singles,
            tc.tile_pool(name="epool", bufs=6) as epool,
            tc.tile_pool(name="otpool", bufs=4) as otpool,
            tc.tile_pool(name="outtp", bufs=3) as outtp,
            tc.tile_pool(name="rcpool", bufs=4) as rcpool,
            tc.tile_pool(name="ps_sc", bufs=2, space="PSUM") as ps_sc,
            tc.tile_pool(name="ps_av", bufs=2, space="PSUM") as ps_av,
            tc.tile_pool(name="ps_aux", bufs=2, space="PSUM") as ps_aux,
        ):
            # ---- resident tensors -------------------------------------
            xt_sb = singles.tile([P, KC, S], BF16, name="xt_sb")
            w_sb = singles.tile([P, KC, 3 * P], BF16, name="w_sb")
            qt = singles.tile([P, S], BF16, name="qt")
            kt = singles.tile([P, S], BF16, name="kt")
            # V per j-tile: [jt, 0:64] head0, [jt, 64] ones, [jt, 65:129]
            # head1, [jt, 129] ones
            v_sb = singles.tile([P, NJ, 130], BF16, name="v_sb")
            masks_f = singles.tile([P, 4, IT], F32, name="masks_f")
            masks = singles.tile([P, 4, IT], BF16, name="masks")
            ident = singles.tile([P, P], F32, name="ident")
            ident_b = singles.tile([P, P], BF16, name="ident_b")
            zbias = singles.tile([P, 1], F32, name="zbias")
            warm = singles.tile([P, 1], F32, name="warm")

            # ---- loads + constants ------------------------------------
            nc.sync.dma_start(
                out=w_sb, in_=w[:, :].rearrange("(c p) n -> p c n", p=P)
            )
            # xt in 8 column blocks so compute can start after block 0
            xt_r = xt[:, :].rearrange("(c p) s -> p c s", p=P)
            for blk in range(NI):
                sl = slice(blk * IT, (blk + 1) * IT)
                nc.sync.dma_start(out=xt_sb[:, :, sl], in_=xt_r[:, :, sl])

            make_identity(nc, ident)
            nc.vector.memset(zbias, 0.0)
            # ones columns for the denominator rows
            nc.vector.memset(v_sb[:, :, 64:65], 1.0)
            nc.vector.memset(v_sb[:, :, 129:130], 1.0)
            # mask k: keep (=1) iff x - p - 128k >= 0, else 0
            for k in range(4):
                nc.gpsimd.memset(masks_f[:, k, :], 1.0)
                nc.gpsimd.affine_select(
                    out=masks_f[:, k, :],
                    in_=masks_f[:, k, :],
                    compare_op=mybir.AluOpType.is_ge,
                    fill=0.0,
                    base=-JT * k,
                    pattern=[[1, IT]],
                    channel_multiplier=-1,
                )
            nc.vector.tensor_copy(masks, masks_f)
            nc.vector.tensor_copy(ident_b, ident)
            # preload the Exp activation table off the critical path
            exp_f = mybir.ActivationFunctionType.Exp
            nc.scalar.activation(warm, zbias, exp_f, bias=zbias, scale=1.0)
            # PE warm-up: the HAM clock gate holds the PE at 1.2 GHz until
            # ~3.4us of sustained activity; burn that window on dummy
            # matmuls while the xt DMA streams so the first projections
            # run at 2.4 GHz
            for _ in range(8):
                hp = ps_sc.tile([P, 2 * IT], F32, tag="sc", name="heat")
                nc.tensor.matmul(
                    hp[:, 0:IT], lhsT=ident_b, rhs=masks[:, 0, :],
                    start=True, stop=True,
                )

            # ---- projections ------------------------------------------
            # Q/K for range r: out = w_chunk.T @ xt_chunk -> [128 dims, 512]
            def project_qk(r):
                sl = slice(r * IT, (r + 1) * IT)
                for half, dst in ((0, qt), (1, kt)):
                    ps_q = ps_aux.tile([P, IT], F32, tag="aux", name="ps_q")
                    for c in range(KC):
                        nc.tensor.matmul(
                            ps_q,
                            lhsT=w_sb[:, c, half * P : (half + 1) * P],
                            rhs=xt_sb[:, c, sl],
                            start=(c == 0),
                            stop=(c == KC - 1),
                        )
                    nc.vector.tensor_copy(dst[:, sl], ps_q)

            # V projection for one 128-row j-tile (small-N matmuls)
            def project_v(j):
                ps_v = ps_aux.tile([P, IT], F32, tag="aux", name="ps_v")
                for c in range(KC):
                    nc.tensor.matmul(
                        ps_v[:, 0:P],
                        lhsT=xt_sb[:, c, j * JT : (j + 1) * JT],
                        rhs=w_sb[:, c, 2 * P : 3 * P],
                        start=(c == 0),
                        stop=(c == KC - 1),
                    )
                nc.vector.tensor_copy(v_sb[:, j, 0:64], ps_v[:, 0:64])
                nc.vector.tensor_copy(v_sb[:, j, 65:129], ps_v[:, 64:128])

            # ---- attention --------------------------------------------
            def tail_block(ot_tiles, ti0, blk):
                # transpose O.T block back to natural layout, normalize by
                # the denominator column, store full 512B rows
                out_t = outtp.tile([P, NHC * HD], F32, tag="outt",
                                   name="out_t")
                for h in range(NHC):
                    tr = ps_aux.tile([P, 65], BF16, tag="aux", name="tr")
                    nc.tensor.transpose(
                        tr, ot_tiles[h][:, blk * P : (blk + 1) * P],
                        ident_b[0:65, 0:65],
                    )
                    rc = rcpool.tile([P, 1], F32, tag="rc", name="rc")
                    nc.vector.reciprocal(rc, tr[:, 64:65])
                    nc.vector.tensor_scalar_mul(
                        out_t[:, h * HD : (h + 1) * HD], tr[:, 0:64], rc
                    )
                nc.sync.dma_start(
                    out=out[ti0 + blk * P : ti0 + (blk + 1) * P, :],
                    in_=out_t,
                )

            # software-pipelined attention: scores for pair p+1 are emitted
            # before exp/AV of pair p, so the PE streams scores while the
            # scalar engine exponentiates the previous pair
            def pair_meta(p_i, njt):
                offs = []
                for u in (0, 1):
                    k = 2 * p_i + u - (njt - 4)
                    offs.append(JT * k if k > 0 else 0)
                return offs, 2 * p_i >= njt - 4

            def emit_scores(p_i, i0, njt):
                offs, diag = pair_meta(p_i, njt)
                sc = [
                    ps_sc.tile([P, 2 * IT], F32, tag="sc", name=f"sc{h}")
                    for h in range(NHC)
                ]
                e = [
                    epool.tile([P, 2 * IT], BF16, tag="e", name=f"e{h}")
                    for h in range(NHC)
                ]
                for u in (0, 1):
                    j = 2 * p_i + u
                    for h in range(NHC):
                        hsl = slice(64 * h, 64 * (h + 1))
                        nc.tensor.matmul(
                            sc[h][:, u * IT + offs[u] : (u + 1) * IT],
                            lhsT=kt[hsl, j * JT : (j + 1) * JT],
                            rhs=qt[hsl, i0 + offs[u] : i0 + IT],
                            start=True,
                            stop=True,
                            tile_position=(64 * h, 0),
                        )
                return sc, e, offs, diag

            def emit_exps(state, p_i, njt):
                sc, e, offs, diag = state
                # on alternating non-diag pairs, head 1's exp runs on the
                # vector engine as a Schraudolph bit-trick (bf16 bits =
                # round(score*A + B) computed as a uint16 tensor_scalar),
                # freeing ScalarE so it is never the pacer
                offload = (not diag) and (p_i % 2 == 0)
                for h in range(NHC):
                    if h == 1 and offload:
                        nc.vector.tensor_scalar(
                            out=e[h].bitcast(mybir.dt.uint16),
                            in0=sc[h],
                            scalar1=SCH_A,
                            scalar2=SCH_B,
                            op0=mybir.AluOpType.mult,
                            op1=mybir.AluOpType.add,
                        )
                    elif not diag:
                        nc.scalar.activation(
                            e[h], sc[h], exp_f, bias=zbias, scale=SCALE
                        )
                    else:
                        for u in (0, 1):
                            k = 2 * p_i + u - (njt - 4)
                            usl = slice(u * IT + offs[u], (u + 1) * IT)
                            nc.scalar.activation(
                                e[h][:, usl], sc[h][:, usl], exp_f,
                                bias=zbias, scale=SCALE,
                            )
                            if k >= 0:
                                nc.vector.tensor_mul(
                                    e[h][:, usl], e[h][:, usl],
                                    masks[:, k, offs[u] : IT],
                                )

            def emit_avs(state, av, p_i, njt):
                _, e, offs, _ = state
                for h in range(NHC):
                    for u in (0, 1):
                        j = 2 * p_i + u
                        nc.tensor.matmul(
                            av[h][:, offs[u] : IT],
                            lhsT=v_sb[:, j, 65 * h : 65 * h + 65],
                            rhs=e[h][:, u * IT + offs[u] : (u + 1) * IT],
                            start=(j == 0),
                            stop=(j == njt - 1),
                        )

            pending = None            # (ot tiles, i0) of the previous range
            project_qk(0)
            for t in range(NI):
                i0 = t * IT
                njt = 4 * (t + 1)         # causal: j-tiles 0..njt-1
                npairs = njt // 2
                last_range = t + 1 == NI
                # V-projection jobs, keyed by the p_i whose AVs they follow
                # (emitted after the attention work so the PE never blocks
                # on a projection that waits for an xt DMA block).  Pair p
                # of range t consumes v j-tiles 2p, 2p+1, which must have
                # been emitted by the end of pair p-1.
                vjobs = {}
                if t == 0:
                    vjobs = {0: (2, 4), 1: (4, 8)} if not last_range \
                        else {0: (2, 4)}
                elif not last_range:
                    j0 = 4 * (t + 1)
                    vjobs = {2: (j0, j0 + 2), 3: (j0 + 2, j0 + 4)}
                av = [
                    ps_av.tile([65, IT], F32, tag="av", name=f"av{h}")
                    for h in range(NHC)
                ]
                if t == 0:
                    state = emit_scores(0, i0, njt)
                    project_v(0)
                    project_v(1)
                for p_i in range(npairs):
                    last_pair = p_i + 1 == npairs
                    emit_exps(state, p_i, njt)
                    # next pair's scores first: independent PE work that
                    # streams while ScalarE exponentiates pair p_i.  The
                    # range t+1 Q/K projection is emitted after them (so
                    # it never delays this range's exps) — except when p1
                    # IS the boundary pair (t==0), where it must precede
                    # the emission of range t+1's first scores (program
                    # order is semantic order for the dependency tracker).
                    if not last_pair:
                        nstate = emit_scores(p_i + 1, i0, njt)
                        if p_i == 1 and not last_range:
                            project_qk(t + 1)
                    elif not last_range:
                        if p_i == 1:
                            project_qk(t + 1)
                        nstate = emit_scores(0, i0 + IT, 4 * (t + 2))
                    else:
                        nstate = None
                    if pending is not None and p_i < IT // P:
                        tail_block(pending[0], pending[1], p_i)
                        if p_i == IT // P - 1:
                            pending = None
                    emit_avs(state, av, p_i, njt)
                    if p_i in vjobs:
                        for j in range(*vjobs[p_i]):
                            project_v(j)
                    state = nstate
                ot = [
                    otpool.tile([65, IT], BF16, tag="ot", name=f"ot{h}")
                    for h in range(NHC)
                ]
                if not last_range:
                    # copy O.T out of PSUM (frees the av accumulators for
                    # the next range); defer transpose/normalize into the
                    # next range's pair loop
                    for h in range(NHC):
                        nc.vector.tensor_copy(ot[h], av[h])
                    pending = (ot, i0)
                else:
                    # final range: per-block copies so each tail block
                    # starts as soon as its slice is staged
                    for blk in range(IT // P):
                        for h in range(NHC):
                            nc.vector.tensor_copy(
                                ot[h][:, blk * P : (blk + 1) * P],
                                av[h][:, blk * P : (blk + 1) * P],
                            )
                        tail_block(ot, i0, blk)
    return nc


def legalize_waits(nc):
    """This toolchain's walrus allows at most ONE sync-wait per instruction;
    split extra waits onto preceding same-engine NoOps (same trick Tile uses
    for its own wait/update carriers)."""
    nsplit = 0
    for f in nc.m.functions:
        for blk in f.blocks:
            new_insts = []
            for inst in blk.instructions:
                si = getattr(inst, "sync_info", None)
                ow = list(si.on_wait) if (si is not None and si.on_wait) else []
                if len(ow) > 1:
                    for w_i, wcond in enumerate(ow[:-1]):
                        nsplit += 1
                        nop = mybir.InstNoOp(
                            name=f"{inst.name}-wsplit{w_i}",
                            sync_info=mybir.SyncInfo(on_wait=[wcond], on_update=[]),
                            bass_nofuse=True,
                            engine=inst.engine,
                        )
                        new_insts.append(nop)
                    si.on_wait = ow[-1:]
                new_insts.append(inst)
            try:
                blk.instructions[:] = new_insts
            except TypeError:
                blk.instructions = new_insts
    return nsplit


_NC_CACHE = None


def _get_nc():
    global _NC_CACHE
    if _NC_CACHE is None:
        nc = build_nc()
        legalize_waits(nc)
        _NC_CACHE = nc
    return _NC_CACHE


def shard_inputs(inputs, qkv_weights):
    import ml_dtypes

    bf16 = ml_dtypes.bfloat16
    x = np.ascontiguousarray(np.asarray(inputs, dtype=np.float32))
    wf = np.ascontiguousarray(np.asarray(qkv_weights, dtype=np.float32))
    in_maps = []
    for c in range(8):
        b, g = divmod(c, 4)
        lo = g * P
        xt_c = np.ascontiguousarray(x[b].T).astype(bf16)
        w_c = np.ascontiguousarray(
            np.concatenate(
                [wf[:, q * D + lo : q * D + lo + P] for q in range(3)], axis=1
            )
        ).astype(bf16)
        in_maps.append({"xt": xt_c, "w": w_c})
    return in_maps


def gather_outputs(results):
    out = np.empty((B, S, D), dtype=np.float32)
    for c in range(8):
        b, g = divmod(c, 4)
        out[b, :, g * P : (g + 1) * P] = results[c]["out"]
    return out


def run(in_maps, **kwargs):
    from concourse.bass_utils import run_bass_kernel_spmd

    return run_bass_kernel_spmd(_get_nc(), in_maps, list(range(8)), **kwargs)


def kernel(**inputs):
    in_maps = shard_inputs(inputs["inputs"], inputs["qkv_weights"])
    res = run(in_maps)
    return gather_outputs(res.results)
